# revision 31
# baseline (speedup 1.0000x reference)
"""Trainium2 Bass kernel for NoisyMixtureOfExperts (top-2 of 8 experts).

Contract: kernel(**inputs) takes the FULL fp32 inputs
  x [4,64,64,768], noise [16384,8], gate_w [8,768], gate_b [8],
  expert_w [8,768,768], expert_b [8,768]
and returns the full output [4,64,64,768] fp32.

Strategy: data-parallel over tokens across 8 NeuronCores (2048 tokens/core,
weights replicated). Per core:
  - gating scores via exact-fp32 PE matmul (top-2 selection fidelity),
    softmax + top-2 masking on DVE/ACT,
  - expert compute with fp32r PE matmuls (full-rate, ~1.5e-4 rel err),
  - VARIANT "dense": all 8 experts for every token, combined with masked
    gate weights,
  - VARIANT "sparse": tokens compacted per expert on-device (cumsum-scan +
    indirect DMA scatter/gather of row indices), per-expert gathered
    matmul, scaled rows scatter-added into the output.

Host-side work is limited to layout transforms of the inputs (shard/
transpose/replicate) and concatenation of per-core outputs.
"""

import os
import sys

sys.path.insert(0, "/opt/trn_rl_repo")
import warnings

warnings.filterwarnings("ignore")

import numpy as np

from concourse import bacc, bass, masks, mybir, tile
from concourse.bass import IndirectOffsetOnAxis
from concourse.bass_utils import run_bass_kernel_spmd

I16 = mybir.dt.int16
F32 = mybir.dt.float32
F32R = mybir.dt.float32r
I32 = mybir.dt.int32
AX = mybir.AxisListType
OP = mybir.AluOpType
ACT = mybir.ActivationFunctionType

NOISE_SCALE = 0.1
N_CORES = 8
D = 768
F = 768
E = 8
N_TOK = 16384
NT = N_TOK // N_CORES          # 2048 tokens per core
TILES = NT // 128              # 16 token tiles per core
KC = D // 128                  # 6 contraction chunks
FH = 2                         # f-dim halves for psum (384 each)
FHW = F // FH

# sparse routing capacity: per-expert token slots (multiple of 128).
# E[count] = 2*NT/E = 512, sigma ~ 21 -> 640 is ~6 sigma.
CAP = 640
CT = CAP // 128                # tiles per expert
BIG = 1.0e6                    # OOB sentinel for padded slots

VARIANT = "dense"              # "dense" | "sparse"
KDEBUG = int(os.environ.get("KDEBUG", "0"))


def _gating(nc, sb, ps, xt_sb, gwt, noise_sb, gateb_sb, g_sb, have_gate_b):
    """Compute masked top-2 gate weights g_sb [128, TILES, E] (0 if unselected)."""
    scratch = sb.tile([128, TILES, E], F32, tag="gat_scratch")
    for t in range(TILES):
        psc = ps.tile([128, E], F32, tag="ps_score")
        for c in range(KC):
            nc.tensor.matmul(
                psc[:],
                xt_sb[:, c, t * 128:(t + 1) * 128],
                gwt[:, c, :],
                start=(c == 0),
                stop=(c == KC - 1),
            )
        s = scratch[:, t, :]
        # s = psc + NOISE_SCALE*noise (+ gate_b)
        nc.vector.scalar_tensor_tensor(
            out=s, in0=noise_sb[:, t, :], scalar=NOISE_SCALE,
            in1=psc[:], op0=OP.mult, op1=OP.add,
        )
        if have_gate_b:
            nc.vector.tensor_tensor(
                out=s, in0=s, in1=gateb_sb[0:1, :].partition_broadcast(128), op=OP.add
            )
        # softmax over E (free dim)
        neg_mx = sb.tile([128, 1], F32, tag="gat_mx")
        nc.vector.tensor_reduce(
            out=neg_mx[:], in_=s, axis=AX.X, op=OP.max, negate=True
        )
        ex = g_sb[:, t, :]
        zs = sb.tile([128, 1], F32, tag="gat_z")
        nc.scalar.activation(
            out=ex, in_=s, func=ACT.Exp, bias=neg_mx[:], scale=1.0,
            accum_out=zs[:],
        )
        rz = sb.tile([128, 1], F32, tag="gat_rz")
        nc.vector.reciprocal(rz[:], zs[:])
        # p = ex * rz (softmax probs)
        p = scratch[:, t, :]
        nc.vector.tensor_scalar_mul(p, ex, rz[:])
        # top-2 mask
        m1 = sb.tile([128, 1], F32, tag="gat_m1")
        nc.vector.tensor_reduce(out=m1[:], in_=p, axis=AX.X, op=OP.max)
        eq = sb.tile([128, E], F32, tag="gat_eq")
        nc.vector.tensor_scalar(
            out=eq[:], in0=p, scalar1=m1[:], scalar2=None, op0=OP.is_ge
        )
        # pm = p - 2*eq (argmax pushed below everything)
        pm = ex  # reuse
        nc.vector.scalar_tensor_tensor(
            out=pm, in0=eq[:], scalar=-2.0, in1=p, op0=OP.mult, op1=OP.add
        )
        m2 = sb.tile([128, 1], F32, tag="gat_m2")
        nc.vector.tensor_reduce(out=m2[:], in_=pm, axis=AX.X, op=OP.max)
        ge = eq  # reuse: ge = p >= m2 (top-2 incl. argmax)
        nc.vector.tensor_scalar(
            out=ge[:], in0=p, scalar1=m2[:], scalar2=None, op0=OP.is_ge
        )
        # g = p * ge
        nc.vector.tensor_tensor(out=g_sb[:, t, :], in0=p, in1=ge[:], op=OP.mult)


def _build_dense(have_gate_b, have_exp_b):
    nc = bacc.Bacc("TRN2", target_bir_lowering=False, debug=False)
    xt = nc.dram_tensor("xt", [D, NT], F32, kind="ExternalInput")
    noise = nc.dram_tensor("noise", [NT, E], F32, kind="ExternalInput")
    gwt = nc.dram_tensor("gwt", [D, E], F32, kind="ExternalInput")
    gateb = nc.dram_tensor("gateb", [1, E], F32, kind="ExternalInput")
    ew = nc.dram_tensor("ew", [E, D, F], F32, kind="ExternalInput")
    eb = nc.dram_tensor("eb", [E, F], F32, kind="ExternalInput")
    out = nc.dram_tensor("out", [NT, F], F32, kind="ExternalOutput")

    with tile.TileContext(nc) as tc:
        with (
            tc.tile_pool(name="sb", bufs=1) as sb,
            tc.tile_pool(name="wpool", bufs=2) as wpool,
            tc.tile_pool(name="ps", bufs=2, space="PSUM") as ps,
            tc.tile_pool(name="pso", bufs=6, space="PSUM") as pso,
        ):
            # Two x^T loads: an exact-fp32 copy for the gating matmuls (a
            # bitcast of the f32r tile runs the gate matmul at f32r
            # precision on HW and flips top-2 selections) and an f32r-typed
            # copy for the full-rate expert matmuls.
            xt_sb = sb.tile([128, KC, NT], F32)
            nc.sync.dma_start(out=xt_sb[:], in_=xt.rearrange("(c p) n -> p c n", p=128))
            xt_r = sb.tile([128, KC, NT], F32R)
            nc.sync.dma_start(
                out=xt_r[:], in_=xt.rearrange("(c p) n -> p c n", p=128).bitcast(F32R)
            )
            gwt_sb = sb.tile([128, KC, E], F32)
            nc.sync.dma_start(out=gwt_sb[:], in_=gwt.rearrange("(c p) e -> p c e", p=128))
            noise_sb = sb.tile([128, TILES, E], F32)
            nc.sync.dma_start(
                out=noise_sb[:], in_=noise.rearrange("(t p) e -> p t e", p=128)
            )
            gateb_sb = sb.tile([1, E], F32)
            nc.sync.dma_start(out=gateb_sb[:], in_=gateb[:])
            eb_sb = sb.tile([E, F], F32)
            nc.sync.dma_start(out=eb_sb[:], in_=eb[:])

            g_sb = sb.tile([128, TILES, E], F32)
            _gating(nc, sb, ps, xt_sb, gwt_sb, noise_sb, gateb_sb, g_sb, have_gate_b)

            acc = sb.tile([128, TILES, F], F32)
            nc.vector.memset(acc[:], 0.0)

            for e in range(E):
                w_sb = wpool.tile([128, KC, F], F32R, tag="w")
                nc.sync.dma_start(
                    out=w_sb[:],
                    in_=ew[e].rearrange("(c p) f -> p c f", p=128).bitcast(F32R),
                )
                for t in range(TILES):
                    for fh in range(FH):
                        po = pso.tile([128, FHW], F32, tag="po")
                        for c in range(KC):
                            nc.tensor.matmul(
                                po[:],
                                xt_r[:, c, t * 128:(t + 1) * 128],
                                w_sb[:, c, fh * FHW:(fh + 1) * FHW],
                                start=(c == 0),
                                stop=(c == KC - 1),
                            )
                        dst = acc[:, t, fh * FHW:(fh + 1) * FHW]
                        if have_exp_b:
                            tmp = sb.tile([128, FHW], F32, tag="btmp")
                            nc.vector.tensor_tensor(
                                out=tmp[:], in0=po[:],
                                in1=eb_sb[e:e + 1, fh * FHW:(fh + 1) * FHW].partition_broadcast(128),
                                op=OP.add,
                            )
                            src = tmp[:]
                        else:
                            src = po[:]
                        # acc += g[:,t,e] * src
                        nc.vector.scalar_tensor_tensor(
                            out=dst, in0=src, scalar=g_sb[:, t, e:e + 1],
                            in1=dst, op0=OP.mult, op1=OP.add,
                        )

            nc.sync.dma_start(
                out=out.rearrange("(t p) f -> p t f", p=128), in_=acc[:]
            )

    nc.compile()
    return nc


def _build_sparse(have_gate_b, have_exp_b):
    nc = bacc.Bacc("TRN2", target_bir_lowering=False, debug=False)
    xt = nc.dram_tensor("xt", [D, NT], F32, kind="ExternalInput")
    xrows = nc.dram_tensor("xrows", [NT, D], F32, kind="ExternalInput")
    noise = nc.dram_tensor("noise", [NT, E], F32, kind="ExternalInput")
    gwt = nc.dram_tensor("gwt", [D, E], F32, kind="ExternalInput")
    gateb = nc.dram_tensor("gateb", [1, E], F32, kind="ExternalInput")
    ew = nc.dram_tensor("ew", [E, D, F], F32, kind="ExternalInput")
    eb = nc.dram_tensor("eb", [E, F], F32, kind="ExternalInput")
    out = nc.dram_tensor("out", [NT, F], F32, kind="ExternalOutput")
    # DRAM scratch. Declared ExternalOutput (not Internal): the 8-core PJRT
    # load may share Internal DRAM across cores. Sized with a junk-pad slot
    # so every scattered index is valid (data-dependent skipped indices
    # corrupt DMA semaphore accounting on HW). Indirect DMA also needs
    # offset-0 tensor bases, hence dedicated tensors.
    g_dram = nc.dram_tensor("g_scratch", [NT + 128, 64], F32,
                            kind="ExternalOutput")
    if KDEBUG:
        dbg_g = nc.dram_tensor("dbg_g", [128, TILES * E], F32, kind="ExternalOutput")
        dbg_idx = nc.dram_tensor("dbg_idx", [1, E * CAP], I32, kind="ExternalOutput")
        dbg_idx16 = nc.dram_tensor("dbg_idx16", [128, E * (CAP // 16)], I32,
                                   kind="ExternalOutput")
        dbg_gv = nc.dram_tensor("dbg_gv", [128, E * CT], F32, kind="ExternalOutput")
        dbg_destA = nc.dram_tensor("dbg_destA", [E, NT], I32, kind="ExternalOutput")
    idxbuf0 = nc.dram_tensor("idx0_scratch", [E * CAP + 16, 1], I16,
                             kind="ExternalOutput")
    idxbufg = nc.dram_tensor("idxg_scratch", [E * CAP + 16, 1], I16,
                             kind="ExternalOutput")
    idxb32a = nc.dram_tensor("idx32a_scratch", [E * CAP + 16, 1], I32,
                             kind="ExternalOutput")
    idxb32g = nc.dram_tensor("idx32g_scratch", [E * CAP + 16, 1], I32,
                             kind="ExternalOutput")

    with tile.TileContext(nc) as tc:
        with (
            tc.tile_pool(name="sb", bufs=1) as sb,
            tc.tile_pool(name="ps", bufs=2, space="PSUM") as ps,
            tc.tile_pool(name="pst", bufs=2, space="PSUM") as pst,
            tc.tile_pool(name="pso", bufs=4, space="PSUM") as pso,
        ):
            # ---------- constants / small loads ----------
            gwt_sb = sb.tile([128, KC, E], F32)
            nc.sync.dma_start(out=gwt_sb[:], in_=gwt.rearrange("(c p) e -> p c e", p=128))
            noise_sb = sb.tile([128, TILES, E], F32)
            nc.sync.dma_start(
                out=noise_sb[:], in_=noise.rearrange("(t p) e -> p t e", p=128)
            )
            gateb_sb = sb.tile([1, E], F32)
            nc.sync.dma_start(out=gateb_sb[:], in_=gateb[:])
            eb_sb = sb.tile([E, F], F32)
            nc.sync.dma_start(out=eb_sb[:], in_=eb[:])
            ident = sb.tile([128, 128], F32)
            masks.make_identity(nc, ident[:])

            g_sb = sb.tile([128, TILES, E], F32)

            # ---------- gating (x^T tile freed afterwards) ----------
            with tc.tile_pool(name="xtp", bufs=1) as xtp:
                xt_sb = xtp.tile([128, KC, NT], F32)
                nc.sync.dma_start(
                    out=xt_sb[:], in_=xt.rearrange("(c p) n -> p c n", p=128)
                )
                _gating(nc, sb, ps, xt_sb, gwt_sb, noise_sb, gateb_sb, g_sb,
                        have_gate_b)

            # g to DRAM [NT, E] for per-slot gather later
            nc.sync.dma_start(
                out=g_dram[0:NT, 0:E].rearrange("(t p) e -> p t e", p=128),
                in_=g_sb[:],
            )
            gz = sb.tile([128, E], F32)
            nc.vector.memset(gz[:], 0.0)
            nc.sync.dma_start(out=g_dram[NT:NT + 128, 0:E], in_=gz[:])

            # ---------- transpose g -> gT [E, NT] ----------
            route_cm = tc.tile_pool(name="route", bufs=1)
            route = route_cm.__enter__()
            gt_sb = route.tile([E, NT], F32, tag="r1")
            for t in range(TILES):
                pt = pst.tile([128, 128], F32, tag="pt")
                nc.tensor.transpose(pt[:E, :], g_sb[:, t, :], ident[:])
                nc.vector.tensor_copy(gt_sb[:, t * 128:(t + 1) * 128], pt[:E, :])

            # ---------- routing: per-expert compacted slot assignment ----------
            mask = route.tile([E, NT], F32, tag="r2")
            nc.vector.tensor_scalar(
                out=mask[:], in0=gt_sb[:], scalar1=0.0, scalar2=None, op0=OP.is_gt
            )
            csum = route.tile([E, NT], F32, tag="r3")
            nc.vector.tensor_tensor_scan(
                out=csum[:], data0=mask[:], data1=mask[:], initial=0.0,
                op0=OP.add, op1=OP.bypass,
            )
            eoff = sb.tile([E, 1], I32)
            nc.gpsimd.iota(eoff[:], pattern=[[1, 1]], base=0, channel_multiplier=CAP)
            eoff_f = sb.tile([E, 1], F32)
            nc.vector.tensor_copy(eoff_f[:], eoff[:])

            # s0: slot within the expert (0-based); ovf: capacity overflow
            s0 = route.tile([E, NT], F32, tag="r4")
            nc.vector.tensor_scalar_add(s0[:], csum[:], -1.0)
            ovf = route.tile([E, NT], F32, tag="r5")
            nc.vector.tensor_scalar(
                out=ovf[:], in0=csum[:], scalar1=CAP + 0.5, scalar2=None, op0=OP.is_ge
            )

            PAD = float(E * CAP)  # junk-heap slot: all masked/overflow writes
                                  # collide here so no index is ever skipped
                                  # (data-dependent skips corrupt DMA sem
                                  # accounting on HW)

            def build_dest(pos, dest, dest_int):
                # dest = mask*(pos + CAP*e - PAD) + PAD, overflow -> PAD
                nc.vector.tensor_scalar(
                    out=dest[:], in0=pos[:], scalar1=eoff_f[:], scalar2=-PAD,
                    op0=OP.add, op1=OP.add,
                )
                nc.vector.tensor_tensor(
                    out=dest[:], in0=dest[:], in1=mask[:], op=OP.mult
                )
                nc.vector.tensor_scalar_add(dest[:], dest[:], PAD)
                nc.vector.scalar_tensor_tensor(
                    out=dest[:], in0=ovf[:], scalar=BIG, in1=dest[:],
                    op0=OP.mult, op1=OP.add,
                )
                nc.vector.tensor_scalar(
                    out=dest[:], in0=dest[:], scalar1=PAD, scalar2=None,
                    op0=OP.min,
                )
                nc.vector.tensor_copy(dest_int[:], dest[:])

            # destA: slot order (for the wrapped-16 int16 index buffer)
            destA = route.tile([E, NT], F32, tag="r6")
            destA_i = route.tile([E, NT], I32, tag="r7")
            build_dest(s0, destA, destA_i)

            tok8 = route.tile([E, NT], I32, tag="r3")  # reuse csum slot
            nc.gpsimd.iota(tok8[:], pattern=[[1, NT]], base=0, channel_multiplier=0)

            # Two sentinel flavors: idxbuf (prefill BIG, used by the per-slot
            # g gather so padded slots read nothing and keep gv=0) and
            # idxbuf0 (prefill 0, used by dma_gather/dma_scatter_add: padded
            # slots gather row 0 and scatter-add an all-zero row to out[0]).
            pre0 = sb.tile([1, E * CAP + 16], I32)
            nc.vector.memset(pre0[:], 0)
            nc.sync.dma_start(out=idxb32a.rearrange("n one -> one n"), in_=pre0[:])
            preg = sb.tile([1, E * CAP + 16], I32)
            nc.vector.memset(preg[:], NT)
            nc.sync.dma_start(out=idxb32g.rearrange("n one -> one n"), in_=preg[:])
            nc.gpsimd.indirect_dma_start(
                out=idxb32a[:],
                out_offset=IndirectOffsetOnAxis(ap=destA_i[:], axis=0),
                in_=tok8[:],
                in_offset=None,
                bounds_check=E * CAP,
                oob_is_err=False,
            )
            nc.gpsimd.indirect_dma_start(
                out=idxb32g[:],
                out_offset=IndirectOffsetOnAxis(ap=destA_i[:], axis=0),
                in_=tok8[:],
                in_offset=None,
                bounds_check=E * CAP,
                oob_is_err=False,
            )
            # Fence the scatters: a full-width SWDGE readback sprays all 16
            # DMA rings, so its (reliable) completion implies every scatter
            # descriptor ahead of it drained; chaining it into a pad-slot
            # write makes later readers of the tensor data-ordered behind it.
            rb0 = sb.tile([1, E * CAP + 16], I16)
            nc.gpsimd.dma_start(
                out=rb0[:],
                in_=bass.AP(idxb32a, 0, [[0, 1], [1, E * CAP + 16]]),
            )
            nc.gpsimd.dma_start(
                out=idxbuf0.rearrange("n one -> one n"), in_=rb0[:]
            )
            rbg = sb.tile([1, E * CAP + 16], I16)
            nc.gpsimd.dma_start(
                out=rbg[:],
                in_=bass.AP(idxb32g, 0, [[0, 1], [1, E * CAP + 16]]),
            )
            nc.gpsimd.dma_start(
                out=idxbufg.rearrange("n one -> one n"), in_=rbg[:]
            )
            if KDEBUG:
                nc.sync.dma_start(out=dbg_g[:],
                                  in_=g_sb[:].rearrange("p t e -> p (t e)"))
                nc.sync.dma_start(out=dbg_idx[:], in_=idx_sb[:])
                nc.sync.dma_start(out=dbg_destA[:], in_=destA_i[:])
            # int16 wrapped-16 index tile for dma_gather/dma_scatter_add:
            # idx16[16g+q, e, m] = idxbuf0[e*CAP + 16m + q] (replicated over g)
            idx16 = sb.tile([128, E, CAP // 16], I16)
            idx16g = sb.tile([128, E, CAP // 16], I16)
            for e in range(E):
                nc.sync.dma_start(
                    out=idx16[0:16, e, :],
                    in_=bass.AP(idxbuf0, e * CAP, [[1, 16], [16, CAP // 16]]),
                )
                nc.sync.dma_start(
                    out=idx16g[0:16, e, :],
                    in_=bass.AP(idxbufg, e * CAP, [[1, 16], [16, CAP // 16]]),
                )
            for gpart in range(1, 8):
                nc.sync.dma_start(
                    out=idx16[16 * gpart:16 * (gpart + 1), :, :],
                    in_=idx16[0:16, :, :],
                )
                nc.sync.dma_start(
                    out=idx16g[16 * gpart:16 * (gpart + 1), :, :],
                    in_=idx16g[0:16, :, :],
                )
            if KDEBUG:
                idx16_i32 = sb.tile([128, E * (CAP // 16)], I32)
                nc.vector.tensor_copy(
                    idx16_i32[:], idx16[:].rearrange("p e m -> p (e m)")
                )
                nc.sync.dma_start(out=dbg_idx16[:], in_=idx16_i32[:])
            route_cm.__exit__(None, None, None)

            # ---------- zero the output (donation may not alias on the
            # multi-core path, so the buffer can hold stale data) ----------
            zrow = sb.tile([128, F], F32)
            nc.vector.memset(zrow[:], 0.0)
            for t in range(TILES):
                nc.sync.dma_start(
                    out=out[t * 128:(t + 1) * 128, :], in_=zrow[:]
                )

            # ---------- per-expert gathered compute ----------
            gv_stage = None
            if KDEBUG:
                gv_stage = sb.tile([128, E * CT], F32, tag="gv_stage")
            wpool_cm = tc.tile_pool(name="wpool", bufs=2)
            gpool_cm = tc.tile_pool(name="gpool", bufs=2)
            rpool_cm = tc.tile_pool(name="rpool", bufs=2)
            wpool = wpool_cm.__enter__()
            gpool = gpool_cm.__enter__()
            rpool = rpool_cm.__enter__()
            for e in range(E):
                w_sb = wpool.tile([128, KC, F], F32R, tag="w")
                nc.sync.dma_start(
                    out=w_sb[:],
                    in_=ew[e].rearrange("(c p) f -> p c f", p=128).bitcast(F32R),
                )
                # gather x rows for this expert's slots (slot i lands at
                # [i %% 128, i // 128, :]; padded slots re-gather row 0)
                xg = gpool.tile([128, CT, D], F32, tag="xg")
                nc.gpsimd.dma_gather(
                    out_ap=xg[:],
                    in_ap=xrows[:],
                    idxs_ap=idx16[:, e, :],
                    num_idxs=CAP,
                    num_idxs_reg=CAP,
                    elem_size=D,
                )
                # gather per-slot gate rows (padded slots hit the zero row)
                gthr = gpool.tile([128, CT, 64], F32, tag="gthr")
                nc.gpsimd.dma_gather(
                    out_ap=gthr[:],
                    in_ap=g_dram[:],
                    idxs_ap=idx16g[:, e, :],
                    num_idxs=CAP,
                    num_idxs_reg=CAP,
                    elem_size=64,
                )
                if KDEBUG:
                    nc.vector.tensor_copy(
                        gv_stage[:, e * CT:(e + 1) * CT], gthr[:, :, e]
                    )
                rows = rpool.tile([128, CT, F], F32, tag="rows")
                for j in range(CT):
                    # transpose gathered tile j -> xgt [128(d), KC, 128(tok)]
                    xgt = gpool.tile([128, KC, 128], F32R, tag="xgt")
                    for c in range(KC):
                        pt = pst.tile([128, 128], F32, tag="pt")
                        nc.tensor.transpose(
                            pt[:], xg[:, j, c * 128:(c + 1) * 128], ident[:]
                        )
                        nc.vector.tensor_copy(xgt[:, c, :], pt[:])
                    for fh in range(FH):
                        po = pso.tile([128, FHW], F32, tag="po")
                        for c in range(KC):
                            nc.tensor.matmul(
                                po[:],
                                xgt[:, c, :],
                                w_sb[:, c, fh * FHW:(fh + 1) * FHW],
                                start=(c == 0),
                                stop=(c == KC - 1),
                            )
                        dst = rows[:, j, fh * FHW:(fh + 1) * FHW]
                        if have_exp_b:
                            nc.vector.tensor_tensor(
                                out=dst, in0=po[:],
                                in1=eb_sb[e:e + 1, fh * FHW:(fh + 1) * FHW].partition_broadcast(128),
                                op=OP.add,
                            )
                            nc.gpsimd.tensor_scalar_mul(
                                dst, dst, gthr[:, j, e:e + 1]
                            )
                        else:
                            nc.vector.tensor_scalar_mul(
                                dst, po[:], gthr[:, j, e:e + 1]
                            )
                # scatter-add rows into out (runtime pre-zeroes out buffers;
                # padded slots add a zero row to out[0])
                nc.gpsimd.dma_scatter_add(
                    out[:],
                    rows[:],
                    idx16[:, e, :],
                    CAP,
                    CAP,
                    F,
                )
            if KDEBUG:
                nc.sync.dma_start(out=dbg_gv[:], in_=gv_stage[:])
            rpool_cm.__exit__(None, None, None)
            gpool_cm.__exit__(None, None, None)
            wpool_cm.__exit__(None, None, None)

    nc.compile()
    return nc


_CACHE = {}


def _get_nc(have_gate_b, have_exp_b):
    key = (VARIANT, have_gate_b, have_exp_b)
    if key not in _CACHE:
        build = _build_dense if VARIANT == "dense" else _build_sparse
        _CACHE[key] = build(have_gate_b, have_exp_b)
    return _CACHE[key]


def _in_maps(x, noise, gate_w, gate_b, expert_w, expert_b):
    x_flat = np.ascontiguousarray(np.asarray(x).reshape(N_TOK, D), dtype=np.float32)
    noise = np.ascontiguousarray(noise, dtype=np.float32)
    gwt = np.ascontiguousarray(np.asarray(gate_w).T, dtype=np.float32)
    gateb = np.ascontiguousarray(np.asarray(gate_b).reshape(1, E), dtype=np.float32)
    ew = np.ascontiguousarray(expert_w, dtype=np.float32)
    eb = np.ascontiguousarray(expert_b, dtype=np.float32)
    maps = []
    for c in range(N_CORES):
        sl = slice(c * NT, (c + 1) * NT)
        m = {
            "xt": np.ascontiguousarray(x_flat[sl].T),
            "noise": noise[sl],
            "gwt": gwt,
            "gateb": gateb,
            "ew": ew,
            "eb": eb,
        }
        if VARIANT == "sparse":
            m["xrows"] = x_flat[sl]
        maps.append(m)
    return maps


def kernel(x, noise, gate_w, gate_b, expert_w, expert_b, _trace=False, **kw):
    have_gate_b = bool(np.any(np.asarray(gate_b)))
    have_exp_b = bool(np.any(np.asarray(expert_b)))
    nc = _get_nc(have_gate_b, have_exp_b)
    maps = _in_maps(x, noise, gate_w, gate_b, expert_w, expert_b)
    res = run_bass_kernel_spmd(nc, maps, core_ids=list(range(N_CORES)), trace=_trace)
    out = np.concatenate([res.results[c]["out"] for c in range(N_CORES)], axis=0)
    if _trace:
        kernel.last_results = res
    return out.reshape(np.asarray(x).shape)


# revision 32
# speedup vs baseline: 1.0505x; 1.0505x over previous
"""Trainium2 Bass kernel for NoisyMixtureOfExperts (top-2 of 8 experts).

Contract: kernel(**inputs) takes the FULL fp32 inputs
  x [4,64,64,768], noise [16384,8], gate_w [8,768], gate_b [8],
  expert_w [8,768,768], expert_b [8,768]
and returns the full output [4,64,64,768] fp32.

Strategy: data-parallel over tokens across 8 NeuronCores (2048 tokens/core,
weights replicated). Per core:
  - gating scores via exact-fp32 PE matmul (top-2 selection fidelity),
    softmax + top-2 masking on DVE/ACT,
  - expert compute with fp32r PE matmuls (full-rate, ~1.5e-4 rel err),
  - VARIANT "dense": all 8 experts for every token, combined with masked
    gate weights,
  - VARIANT "sparse": tokens compacted per expert on-device (cumsum-scan +
    indirect DMA scatter/gather of row indices), per-expert gathered
    matmul, scaled rows scatter-added into the output.

Host-side work is limited to layout transforms of the inputs (shard/
transpose/replicate) and concatenation of per-core outputs.
"""

import os
import sys

sys.path.insert(0, "/opt/trn_rl_repo")
import warnings

warnings.filterwarnings("ignore")

import numpy as np

from concourse import bacc, bass, masks, mybir, tile
from concourse.bass import IndirectOffsetOnAxis
from concourse.bass_utils import run_bass_kernel_spmd

I16 = mybir.dt.int16
F32 = mybir.dt.float32
F32R = mybir.dt.float32r
I32 = mybir.dt.int32
AX = mybir.AxisListType
OP = mybir.AluOpType
ACT = mybir.ActivationFunctionType

NOISE_SCALE = 0.1
N_CORES = 8
D = 768
F = 768
E = 8
N_TOK = 16384
NT = N_TOK // N_CORES          # 2048 tokens per core
TILES = NT // 128              # 16 token tiles per core
KC = D // 128                  # 6 contraction chunks
FH = 2                         # f-dim halves for psum (384 each)
FHW = F // FH

# sparse routing capacity: per-expert token slots (multiple of 128).
# E[count] = 2*NT/E = 512, sigma ~ 21 -> 640 is ~6 sigma.
CAP = 640
CT = CAP // 128                # tiles per expert
BIG = 1.0e6                    # OOB sentinel for padded slots

VARIANT = "dense"              # "dense" | "sparse"
KDEBUG = int(os.environ.get("KDEBUG", "0"))


def _gating(nc, sb, ps, xt_sb, gwt, noise_sb, gateb_sb, g_sb, have_gate_b):
    """Compute masked top-2 gate weights g_sb [128, TILES, E] (0 if unselected)."""
    scratch = sb.tile([128, TILES, E], F32, tag="gat_scratch")
    for t in range(TILES):
        psc = ps.tile([128, E], F32, tag="ps_score")
        for c in range(KC):
            nc.tensor.matmul(
                psc[:],
                xt_sb[:, c, t * 128:(t + 1) * 128],
                gwt[:, c, :],
                start=(c == 0),
                stop=(c == KC - 1),
            )
        s = scratch[:, t, :]
        # s = psc + NOISE_SCALE*noise (+ gate_b)
        nc.vector.scalar_tensor_tensor(
            out=s, in0=noise_sb[:, t, :], scalar=NOISE_SCALE,
            in1=psc[:], op0=OP.mult, op1=OP.add,
        )
        if have_gate_b:
            nc.vector.tensor_tensor(
                out=s, in0=s, in1=gateb_sb[0:1, :].partition_broadcast(128), op=OP.add
            )
        # softmax over E (free dim)
        neg_mx = sb.tile([128, 1], F32, tag="gat_mx")
        nc.vector.tensor_reduce(
            out=neg_mx[:], in_=s, axis=AX.X, op=OP.max, negate=True
        )
        ex = g_sb[:, t, :]
        zs = sb.tile([128, 1], F32, tag="gat_z")
        nc.scalar.activation(
            out=ex, in_=s, func=ACT.Exp, bias=neg_mx[:], scale=1.0,
            accum_out=zs[:],
        )
        rz = sb.tile([128, 1], F32, tag="gat_rz")
        nc.vector.reciprocal(rz[:], zs[:])
        # p = ex * rz (softmax probs)
        p = scratch[:, t, :]
        nc.vector.tensor_scalar_mul(p, ex, rz[:])
        # top-2 mask
        m1 = sb.tile([128, 1], F32, tag="gat_m1")
        nc.vector.tensor_reduce(out=m1[:], in_=p, axis=AX.X, op=OP.max)
        eq = sb.tile([128, E], F32, tag="gat_eq")
        nc.vector.tensor_scalar(
            out=eq[:], in0=p, scalar1=m1[:], scalar2=None, op0=OP.is_ge
        )
        # pm = p - 2*eq (argmax pushed below everything)
        pm = ex  # reuse
        nc.vector.scalar_tensor_tensor(
            out=pm, in0=eq[:], scalar=-2.0, in1=p, op0=OP.mult, op1=OP.add
        )
        m2 = sb.tile([128, 1], F32, tag="gat_m2")
        nc.vector.tensor_reduce(out=m2[:], in_=pm, axis=AX.X, op=OP.max)
        ge = eq  # reuse: ge = p >= m2 (top-2 incl. argmax)
        nc.vector.tensor_scalar(
            out=ge[:], in0=p, scalar1=m2[:], scalar2=None, op0=OP.is_ge
        )
        # g = p * ge
        nc.vector.tensor_tensor(out=g_sb[:, t, :], in0=p, in1=ge[:], op=OP.mult)


def _build_dense(have_gate_b, have_exp_b):
    nc = bacc.Bacc("TRN2", target_bir_lowering=False, debug=False)
    xt = nc.dram_tensor("xt", [D, NT], F32, kind="ExternalInput")
    noise = nc.dram_tensor("noise", [NT, E], F32, kind="ExternalInput")
    gwt = nc.dram_tensor("gwt", [D, E], F32, kind="ExternalInput")
    gateb = nc.dram_tensor("gateb", [1, E], F32, kind="ExternalInput")
    ew = nc.dram_tensor("ew", [E, D, F], F32, kind="ExternalInput")
    eb = nc.dram_tensor("eb", [E, F], F32, kind="ExternalInput")
    out = nc.dram_tensor("out", [NT, F], F32, kind="ExternalOutput")

    with tile.TileContext(nc) as tc:
        with (
            tc.tile_pool(name="sb", bufs=1) as sb,
            tc.tile_pool(name="wpool", bufs=2) as wpool,
            tc.tile_pool(name="ps", bufs=2, space="PSUM") as ps,
            tc.tile_pool(name="pso", bufs=6, space="PSUM") as pso,
        ):
            # One exact-fp32 x^T load for the gating matmuls (bitcasting an
            # f32r tile for gating runs the gate matmul at f32r precision on
            # HW and flips top-2 selections); the expert-matmul f32r copy is
            # made on-chip with a DVE cast-copy instead of a second 6.3MB
            # DMA load.
            xt_sb = sb.tile([128, KC, NT], F32)
            nc.sync.dma_start(out=xt_sb[:], in_=xt.rearrange("(c p) n -> p c n", p=128))
            xt_r = sb.tile([128, KC, NT], F32R)
            for c in range(KC):
                nc.vector.tensor_copy(xt_r[:, c, :], xt_sb[:, c, :])
            gwt_sb = sb.tile([128, KC, E], F32)
            nc.sync.dma_start(out=gwt_sb[:], in_=gwt.rearrange("(c p) e -> p c e", p=128))
            noise_sb = sb.tile([128, TILES, E], F32)
            nc.sync.dma_start(
                out=noise_sb[:], in_=noise.rearrange("(t p) e -> p t e", p=128)
            )
            gateb_sb = sb.tile([1, E], F32)
            nc.sync.dma_start(out=gateb_sb[:], in_=gateb[:])
            eb_sb = sb.tile([E, F], F32)
            nc.sync.dma_start(out=eb_sb[:], in_=eb[:])

            g_sb = sb.tile([128, TILES, E], F32)
            _gating(nc, sb, ps, xt_sb, gwt_sb, noise_sb, gateb_sb, g_sb, have_gate_b)

            acc = sb.tile([128, TILES, F], F32)
            nc.vector.memset(acc[:], 0.0)

            for e in range(E):
                w_sb = wpool.tile([128, KC, F], F32R, tag="w")
                nc.sync.dma_start(
                    out=w_sb[:],
                    in_=ew[e].rearrange("(c p) f -> p c f", p=128).bitcast(F32R),
                )
                for t in range(TILES):
                    for fh in range(FH):
                        po = pso.tile([128, FHW], F32, tag="po")
                        for c in range(KC):
                            nc.tensor.matmul(
                                po[:],
                                xt_r[:, c, t * 128:(t + 1) * 128],
                                w_sb[:, c, fh * FHW:(fh + 1) * FHW],
                                start=(c == 0),
                                stop=(c == KC - 1),
                            )
                        dst = acc[:, t, fh * FHW:(fh + 1) * FHW]
                        if have_exp_b:
                            tmp = sb.tile([128, FHW], F32, tag="btmp")
                            nc.vector.tensor_tensor(
                                out=tmp[:], in0=po[:],
                                in1=eb_sb[e:e + 1, fh * FHW:(fh + 1) * FHW].partition_broadcast(128),
                                op=OP.add,
                            )
                            src = tmp[:]
                        else:
                            src = po[:]
                        # acc += g[:,t,e] * src
                        nc.vector.scalar_tensor_tensor(
                            out=dst, in0=src, scalar=g_sb[:, t, e:e + 1],
                            in1=dst, op0=OP.mult, op1=OP.add,
                        )

            nc.sync.dma_start(
                out=out.rearrange("(t p) f -> p t f", p=128), in_=acc[:]
            )

    nc.compile()
    return nc


def _build_sparse(have_gate_b, have_exp_b):
    nc = bacc.Bacc("TRN2", target_bir_lowering=False, debug=False)
    xt = nc.dram_tensor("xt", [D, NT], F32, kind="ExternalInput")
    xrows = nc.dram_tensor("xrows", [NT, D], F32, kind="ExternalInput")
    noise = nc.dram_tensor("noise", [NT, E], F32, kind="ExternalInput")
    gwt = nc.dram_tensor("gwt", [D, E], F32, kind="ExternalInput")
    gateb = nc.dram_tensor("gateb", [1, E], F32, kind="ExternalInput")
    ew = nc.dram_tensor("ew", [E, D, F], F32, kind="ExternalInput")
    eb = nc.dram_tensor("eb", [E, F], F32, kind="ExternalInput")
    out = nc.dram_tensor("out", [NT, F], F32, kind="ExternalOutput")
    # DRAM scratch. Declared ExternalOutput (not Internal): the 8-core PJRT
    # load may share Internal DRAM across cores. Sized with a junk-pad slot
    # so every scattered index is valid (data-dependent skipped indices
    # corrupt DMA semaphore accounting on HW). Indirect DMA also needs
    # offset-0 tensor bases, hence dedicated tensors.
    g_dram = nc.dram_tensor("g_scratch", [NT + 128, 64], F32,
                            kind="ExternalOutput")
    if KDEBUG:
        dbg_g = nc.dram_tensor("dbg_g", [128, TILES * E], F32, kind="ExternalOutput")
        dbg_idx = nc.dram_tensor("dbg_idx", [1, E * CAP], I32, kind="ExternalOutput")
        dbg_idx16 = nc.dram_tensor("dbg_idx16", [128, E * (CAP // 16)], I32,
                                   kind="ExternalOutput")
        dbg_gv = nc.dram_tensor("dbg_gv", [128, E * CT], F32, kind="ExternalOutput")
        dbg_destA = nc.dram_tensor("dbg_destA", [E, NT], I32, kind="ExternalOutput")
    idxbuf0 = nc.dram_tensor("idx0_scratch", [E * CAP + 16, 1], I16,
                             kind="ExternalOutput")
    idxbufg = nc.dram_tensor("idxg_scratch", [E * CAP + 16, 1], I16,
                             kind="ExternalOutput")
    idxb32a = nc.dram_tensor("idx32a_scratch", [E * CAP + 16, 1], I32,
                             kind="ExternalOutput")
    idxb32g = nc.dram_tensor("idx32g_scratch", [E * CAP + 16, 1], I32,
                             kind="ExternalOutput")

    with tile.TileContext(nc) as tc:
        with (
            tc.tile_pool(name="sb", bufs=1) as sb,
            tc.tile_pool(name="ps", bufs=2, space="PSUM") as ps,
            tc.tile_pool(name="pst", bufs=2, space="PSUM") as pst,
            tc.tile_pool(name="pso", bufs=4, space="PSUM") as pso,
        ):
            # ---------- constants / small loads ----------
            gwt_sb = sb.tile([128, KC, E], F32)
            nc.sync.dma_start(out=gwt_sb[:], in_=gwt.rearrange("(c p) e -> p c e", p=128))
            noise_sb = sb.tile([128, TILES, E], F32)
            nc.sync.dma_start(
                out=noise_sb[:], in_=noise.rearrange("(t p) e -> p t e", p=128)
            )
            gateb_sb = sb.tile([1, E], F32)
            nc.sync.dma_start(out=gateb_sb[:], in_=gateb[:])
            eb_sb = sb.tile([E, F], F32)
            nc.sync.dma_start(out=eb_sb[:], in_=eb[:])
            ident = sb.tile([128, 128], F32)
            masks.make_identity(nc, ident[:])

            g_sb = sb.tile([128, TILES, E], F32)

            # ---------- gating (x^T tile freed afterwards) ----------
            with tc.tile_pool(name="xtp", bufs=1) as xtp:
                xt_sb = xtp.tile([128, KC, NT], F32)
                nc.sync.dma_start(
                    out=xt_sb[:], in_=xt.rearrange("(c p) n -> p c n", p=128)
                )
                _gating(nc, sb, ps, xt_sb, gwt_sb, noise_sb, gateb_sb, g_sb,
                        have_gate_b)

            # g to DRAM [NT, E] for per-slot gather later
            nc.sync.dma_start(
                out=g_dram[0:NT, 0:E].rearrange("(t p) e -> p t e", p=128),
                in_=g_sb[:],
            )
            gz = sb.tile([128, E], F32)
            nc.vector.memset(gz[:], 0.0)
            nc.sync.dma_start(out=g_dram[NT:NT + 128, 0:E], in_=gz[:])

            # ---------- transpose g -> gT [E, NT] ----------
            route_cm = tc.tile_pool(name="route", bufs=1)
            route = route_cm.__enter__()
            gt_sb = route.tile([E, NT], F32, tag="r1")
            for t in range(TILES):
                pt = pst.tile([128, 128], F32, tag="pt")
                nc.tensor.transpose(pt[:E, :], g_sb[:, t, :], ident[:])
                nc.vector.tensor_copy(gt_sb[:, t * 128:(t + 1) * 128], pt[:E, :])

            # ---------- routing: per-expert compacted slot assignment ----------
            mask = route.tile([E, NT], F32, tag="r2")
            nc.vector.tensor_scalar(
                out=mask[:], in0=gt_sb[:], scalar1=0.0, scalar2=None, op0=OP.is_gt
            )
            csum = route.tile([E, NT], F32, tag="r3")
            nc.vector.tensor_tensor_scan(
                out=csum[:], data0=mask[:], data1=mask[:], initial=0.0,
                op0=OP.add, op1=OP.bypass,
            )
            eoff = sb.tile([E, 1], I32)
            nc.gpsimd.iota(eoff[:], pattern=[[1, 1]], base=0, channel_multiplier=CAP)
            eoff_f = sb.tile([E, 1], F32)
            nc.vector.tensor_copy(eoff_f[:], eoff[:])

            # s0: slot within the expert (0-based); ovf: capacity overflow
            s0 = route.tile([E, NT], F32, tag="r4")
            nc.vector.tensor_scalar_add(s0[:], csum[:], -1.0)
            ovf = route.tile([E, NT], F32, tag="r5")
            nc.vector.tensor_scalar(
                out=ovf[:], in0=csum[:], scalar1=CAP + 0.5, scalar2=None, op0=OP.is_ge
            )

            PAD = float(E * CAP)  # junk-heap slot: all masked/overflow writes
                                  # collide here so no index is ever skipped
                                  # (data-dependent skips corrupt DMA sem
                                  # accounting on HW)

            def build_dest(pos, dest, dest_int):
                # dest = mask*(pos + CAP*e - PAD) + PAD, overflow -> PAD
                nc.vector.tensor_scalar(
                    out=dest[:], in0=pos[:], scalar1=eoff_f[:], scalar2=-PAD,
                    op0=OP.add, op1=OP.add,
                )
                nc.vector.tensor_tensor(
                    out=dest[:], in0=dest[:], in1=mask[:], op=OP.mult
                )
                nc.vector.tensor_scalar_add(dest[:], dest[:], PAD)
                nc.vector.scalar_tensor_tensor(
                    out=dest[:], in0=ovf[:], scalar=BIG, in1=dest[:],
                    op0=OP.mult, op1=OP.add,
                )
                nc.vector.tensor_scalar(
                    out=dest[:], in0=dest[:], scalar1=PAD, scalar2=None,
                    op0=OP.min,
                )
                nc.vector.tensor_copy(dest_int[:], dest[:])

            # destA: slot order (for the wrapped-16 int16 index buffer)
            destA = route.tile([E, NT], F32, tag="r6")
            destA_i = route.tile([E, NT], I32, tag="r7")
            build_dest(s0, destA, destA_i)

            tok8 = route.tile([E, NT], I32, tag="r3")  # reuse csum slot
            nc.gpsimd.iota(tok8[:], pattern=[[1, NT]], base=0, channel_multiplier=0)

            # Two sentinel flavors: idxbuf (prefill BIG, used by the per-slot
            # g gather so padded slots read nothing and keep gv=0) and
            # idxbuf0 (prefill 0, used by dma_gather/dma_scatter_add: padded
            # slots gather row 0 and scatter-add an all-zero row to out[0]).
            pre0 = sb.tile([1, E * CAP + 16], I32)
            nc.vector.memset(pre0[:], 0)
            nc.sync.dma_start(out=idxb32a.rearrange("n one -> one n"), in_=pre0[:])
            preg = sb.tile([1, E * CAP + 16], I32)
            nc.vector.memset(preg[:], NT)
            nc.sync.dma_start(out=idxb32g.rearrange("n one -> one n"), in_=preg[:])
            nc.gpsimd.indirect_dma_start(
                out=idxb32a[:],
                out_offset=IndirectOffsetOnAxis(ap=destA_i[:], axis=0),
                in_=tok8[:],
                in_offset=None,
                bounds_check=E * CAP,
                oob_is_err=False,
            )
            nc.gpsimd.indirect_dma_start(
                out=idxb32g[:],
                out_offset=IndirectOffsetOnAxis(ap=destA_i[:], axis=0),
                in_=tok8[:],
                in_offset=None,
                bounds_check=E * CAP,
                oob_is_err=False,
            )
            # Fence the scatters: a full-width SWDGE readback sprays all 16
            # DMA rings, so its (reliable) completion implies every scatter
            # descriptor ahead of it drained; chaining it into a pad-slot
            # write makes later readers of the tensor data-ordered behind it.
            rb0 = sb.tile([1, E * CAP + 16], I16)
            nc.gpsimd.dma_start(
                out=rb0[:],
                in_=bass.AP(idxb32a, 0, [[0, 1], [1, E * CAP + 16]]),
            )
            nc.gpsimd.dma_start(
                out=idxbuf0.rearrange("n one -> one n"), in_=rb0[:]
            )
            rbg = sb.tile([1, E * CAP + 16], I16)
            nc.gpsimd.dma_start(
                out=rbg[:],
                in_=bass.AP(idxb32g, 0, [[0, 1], [1, E * CAP + 16]]),
            )
            nc.gpsimd.dma_start(
                out=idxbufg.rearrange("n one -> one n"), in_=rbg[:]
            )
            if KDEBUG:
                nc.sync.dma_start(out=dbg_g[:],
                                  in_=g_sb[:].rearrange("p t e -> p (t e)"))
                nc.sync.dma_start(out=dbg_idx[:], in_=idx_sb[:])
                nc.sync.dma_start(out=dbg_destA[:], in_=destA_i[:])
            # int16 wrapped-16 index tile for dma_gather/dma_scatter_add:
            # idx16[16g+q, e, m] = idxbuf0[e*CAP + 16m + q] (replicated over g)
            idx16 = sb.tile([128, E, CAP // 16], I16)
            idx16g = sb.tile([128, E, CAP // 16], I16)
            for e in range(E):
                nc.sync.dma_start(
                    out=idx16[0:16, e, :],
                    in_=bass.AP(idxbuf0, e * CAP, [[1, 16], [16, CAP // 16]]),
                )
                nc.sync.dma_start(
                    out=idx16g[0:16, e, :],
                    in_=bass.AP(idxbufg, e * CAP, [[1, 16], [16, CAP // 16]]),
                )
            for gpart in range(1, 8):
                nc.sync.dma_start(
                    out=idx16[16 * gpart:16 * (gpart + 1), :, :],
                    in_=idx16[0:16, :, :],
                )
                nc.sync.dma_start(
                    out=idx16g[16 * gpart:16 * (gpart + 1), :, :],
                    in_=idx16g[0:16, :, :],
                )
            if KDEBUG:
                idx16_i32 = sb.tile([128, E * (CAP // 16)], I32)
                nc.vector.tensor_copy(
                    idx16_i32[:], idx16[:].rearrange("p e m -> p (e m)")
                )
                nc.sync.dma_start(out=dbg_idx16[:], in_=idx16_i32[:])
            route_cm.__exit__(None, None, None)

            # ---------- zero the output (donation may not alias on the
            # multi-core path, so the buffer can hold stale data) ----------
            zrow = sb.tile([128, F], F32)
            nc.vector.memset(zrow[:], 0.0)
            for t in range(TILES):
                nc.sync.dma_start(
                    out=out[t * 128:(t + 1) * 128, :], in_=zrow[:]
                )

            # ---------- per-expert gathered compute ----------
            gv_stage = None
            if KDEBUG:
                gv_stage = sb.tile([128, E * CT], F32, tag="gv_stage")
            wpool_cm = tc.tile_pool(name="wpool", bufs=2)
            gpool_cm = tc.tile_pool(name="gpool", bufs=2)
            rpool_cm = tc.tile_pool(name="rpool", bufs=2)
            wpool = wpool_cm.__enter__()
            gpool = gpool_cm.__enter__()
            rpool = rpool_cm.__enter__()
            for e in range(E):
                w_sb = wpool.tile([128, KC, F], F32R, tag="w")
                nc.sync.dma_start(
                    out=w_sb[:],
                    in_=ew[e].rearrange("(c p) f -> p c f", p=128).bitcast(F32R),
                )
                # gather x rows for this expert's slots (slot i lands at
                # [i %% 128, i // 128, :]; padded slots re-gather row 0)
                xg = gpool.tile([128, CT, D], F32, tag="xg")
                nc.gpsimd.dma_gather(
                    out_ap=xg[:],
                    in_ap=xrows[:],
                    idxs_ap=idx16[:, e, :],
                    num_idxs=CAP,
                    num_idxs_reg=CAP,
                    elem_size=D,
                )
                # gather per-slot gate rows (padded slots hit the zero row)
                gthr = gpool.tile([128, CT, 64], F32, tag="gthr")
                nc.gpsimd.dma_gather(
                    out_ap=gthr[:],
                    in_ap=g_dram[:],
                    idxs_ap=idx16g[:, e, :],
                    num_idxs=CAP,
                    num_idxs_reg=CAP,
                    elem_size=64,
                )
                if KDEBUG:
                    nc.vector.tensor_copy(
                        gv_stage[:, e * CT:(e + 1) * CT], gthr[:, :, e]
                    )
                rows = rpool.tile([128, CT, F], F32, tag="rows")
                for j in range(CT):
                    # transpose gathered tile j -> xgt [128(d), KC, 128(tok)]
                    xgt = gpool.tile([128, KC, 128], F32R, tag="xgt")
                    for c in range(KC):
                        pt = pst.tile([128, 128], F32, tag="pt")
                        nc.tensor.transpose(
                            pt[:], xg[:, j, c * 128:(c + 1) * 128], ident[:]
                        )
                        nc.vector.tensor_copy(xgt[:, c, :], pt[:])
                    for fh in range(FH):
                        po = pso.tile([128, FHW], F32, tag="po")
                        for c in range(KC):
                            nc.tensor.matmul(
                                po[:],
                                xgt[:, c, :],
                                w_sb[:, c, fh * FHW:(fh + 1) * FHW],
                                start=(c == 0),
                                stop=(c == KC - 1),
                            )
                        dst = rows[:, j, fh * FHW:(fh + 1) * FHW]
                        if have_exp_b:
                            nc.vector.tensor_tensor(
                                out=dst, in0=po[:],
                                in1=eb_sb[e:e + 1, fh * FHW:(fh + 1) * FHW].partition_broadcast(128),
                                op=OP.add,
                            )
                            nc.gpsimd.tensor_scalar_mul(
                                dst, dst, gthr[:, j, e:e + 1]
                            )
                        else:
                            nc.vector.tensor_scalar_mul(
                                dst, po[:], gthr[:, j, e:e + 1]
                            )
                # scatter-add rows into out (runtime pre-zeroes out buffers;
                # padded slots add a zero row to out[0])
                nc.gpsimd.dma_scatter_add(
                    out[:],
                    rows[:],
                    idx16[:, e, :],
                    CAP,
                    CAP,
                    F,
                )
            if KDEBUG:
                nc.sync.dma_start(out=dbg_gv[:], in_=gv_stage[:])
            rpool_cm.__exit__(None, None, None)
            gpool_cm.__exit__(None, None, None)
            wpool_cm.__exit__(None, None, None)

    nc.compile()
    return nc


_CACHE = {}


def _get_nc(have_gate_b, have_exp_b):
    key = (VARIANT, have_gate_b, have_exp_b)
    if key not in _CACHE:
        build = _build_dense if VARIANT == "dense" else _build_sparse
        _CACHE[key] = build(have_gate_b, have_exp_b)
    return _CACHE[key]


def _in_maps(x, noise, gate_w, gate_b, expert_w, expert_b):
    x_flat = np.ascontiguousarray(np.asarray(x).reshape(N_TOK, D), dtype=np.float32)
    noise = np.ascontiguousarray(noise, dtype=np.float32)
    gwt = np.ascontiguousarray(np.asarray(gate_w).T, dtype=np.float32)
    gateb = np.ascontiguousarray(np.asarray(gate_b).reshape(1, E), dtype=np.float32)
    ew = np.ascontiguousarray(expert_w, dtype=np.float32)
    eb = np.ascontiguousarray(expert_b, dtype=np.float32)
    maps = []
    for c in range(N_CORES):
        sl = slice(c * NT, (c + 1) * NT)
        m = {
            "xt": np.ascontiguousarray(x_flat[sl].T),
            "noise": noise[sl],
            "gwt": gwt,
            "gateb": gateb,
            "ew": ew,
            "eb": eb,
        }
        if VARIANT == "sparse":
            m["xrows"] = x_flat[sl]
        maps.append(m)
    return maps


def kernel(x, noise, gate_w, gate_b, expert_w, expert_b, _trace=False, **kw):
    have_gate_b = bool(np.any(np.asarray(gate_b)))
    have_exp_b = bool(np.any(np.asarray(expert_b)))
    nc = _get_nc(have_gate_b, have_exp_b)
    maps = _in_maps(x, noise, gate_w, gate_b, expert_w, expert_b)
    res = run_bass_kernel_spmd(nc, maps, core_ids=list(range(N_CORES)), trace=_trace)
    out = np.concatenate([res.results[c]["out"] for c in range(N_CORES)], axis=0)
    if _trace:
        kernel.last_results = res
    return out.reshape(np.asarray(x).shape)
